# revision 51
# baseline (speedup 1.0000x reference)
"""Causal attention (B=4, T=4096, D=256) on 8 TRN2 NeuronCores.

Sharding: data-parallel over batch x query-halves. Core c handles batch
b = c//2 and query half h = c%2. The active builder (v4, VERSION=4)
groups queries 512 wide: group g of core h owns the interleaved global
128-row query tiles {8g + 2u + h : u in 0..3}, so both halves see the
same s-extent (8g+8 tiles) per group -- causal work is exactly balanced
and the program is SPMD-uniform (identical instruction stream on every
core; only input DATA differs).

v4 design (flash-attention style, nothing T^2-sized touches HBM):
  - QK^T in fp8e4 via DoubleRow matmuls: ONE matmul per s-tile contracts
    d=256 (two k-interleaved 128-tiles) at 2x the bf16 rate. KT8/QT8
    [128, 2, t] fp8 are written DIRECTLY by the projection PSUM->SBUF
    cast copies (no extra cast pass). K carries no bias: softmax is
    invariant to per-query shifts, so (q+bq)@(k+bk) == (q+bq)@k there.
  - s-tiles processed in PAIRS sharing a [128,2,512] 2-bank PSUM tile
    and ONE wide exp (ACT) per pair; diagonal pairs share the same
    narrowing c0 and take a single paired 0/1 bf16 mask multiply (DVE).
  - 1-stage software-pipelined attention stream: S^T+exp of pair i+1 is
    emitted before the PV block of pair i (also across group borders),
    so the exp latency always hides behind PE work. PV matmuls stay
    j-major/u-ascending -- other emission orders measurably slow every
    matmul (~20%) via SBUF access-pattern interference.
  - O [q, 257] accumulates in 4 single-bank PSUM tiles per group with a
    ones-column denominator; each o[u] is finalized (reciprocal + STT
    with bv folded in) immediately after its stop matmul; y is stored
    in BF16 (host casts back to f32; ~0.02% extra error) batched per
    group, except the last group which stores per-u so the final
    (tail-critical) DMA payload is small.
  - all projections run upfront as single-bank PSUM tiles rotating over
    all 6 PSUM slots, cast copies alternating DVE/ACT; V is projected
    in pairs into one [128, 32, 257] bf16 tensor (bias-free; bv moves
    to the finalize).
  - NO xqT input: the host pair-swaps the x tile axis for h=1 cores, so
    Q-projection reads the strided slice xT[k][:, 8g:8g+8:2, :]
    uniformly; only the mask data (cm) differs per core-half.
  - inputs stream over two HWDGE queues (SP: wk + k=0 chunks, ACT: cf +
    k=1 chunks; GPSIMD: masks) in consumption order; 11 garbage warm-up
    matmuls bridge the DMA wait with the PE clock gate open.
Measured 85.3-86.2 us on hardware (mean ~85.7, 8 cores, run-to-run
drift ~1 us), rel err 1.14e-2 vs the fp32 reference (fp8 QK^T quantization; limit is
2e-2). Baseline v3 (bf16, VERSION=3) measured ~111.4 us. Rejected by
measurement: fp8 K-projection via x8/wk8 DoubleRow (no net time gain --
the extra 1MB x8 load eats the PE saving -- and rel err rises to
1.5e-2); per-j exps everywhere (ACT-bound, +10us); any PV emission
order other than j-major/u-ascending (+3..17us).
"""

import os
import sys

import numpy as np

for _p in ("/opt/trn_rl_repo", "/root/.axon_site/_ro/trn_rl_repo"):
    if os.path.isdir(_p) and _p not in sys.path:
        sys.path.insert(0, _p)

import ml_dtypes  # noqa: E402

import concourse.bass as bass  # noqa: E402
import concourse.bacc as bacc  # noqa: E402
import concourse.mybir as mybir  # noqa: E402
import concourse.tile as tile  # noqa: E402
from concourse.bass_utils import run_bass_kernel_spmd  # noqa: E402

BF16 = mybir.dt.bfloat16
F32 = mybir.dt.float32
NPBF16 = ml_dtypes.bfloat16
NPFP8 = ml_dtypes.float8_e4m3

B = 4
T = 4096
D = 256
N_CORES = 8
TQ = T // 2  # query rows per core
NEG = -1.0e9


def build_nc(t: int = T, tq: int = TQ) -> bass.Bass:
    nq = tq // 128  # query tiles per core
    ns = t // 128  # total key tiles
    assert t == 2 * tq and ns == 2 * nq
    scale = 1.0 / float(np.sqrt(np.float32(D)))

    nc = bacc.Bacc()
    xT_d = nc.dram_tensor("xT", [2, 128, t], BF16, kind="ExternalInput")
    xqT_d = nc.dram_tensor("xqT", [2, 128, tq], BF16, kind="ExternalInput")
    wq_d = nc.dram_tensor("wq", [2, 128, D], BF16, kind="ExternalInput")
    wk_d = nc.dram_tensor("wk", [2, 128, D], BF16, kind="ExternalInput")
    wv_d = nc.dram_tensor("wv", [2, 128, D], BF16, kind="ExternalInput")
    bq_d = nc.dram_tensor("bq", [2, 128, 1], F32, kind="ExternalInput")
    bk_d = nc.dram_tensor("bk", [2, 128, 1], F32, kind="ExternalInput")
    bvb_d = nc.dram_tensor("bvb", [128, D], F32, kind="ExternalInput")
    mask_d = nc.dram_tensor("mask", [128, 256], F32, kind="ExternalInput")
    y_d = nc.dram_tensor("y", [tq, D], F32, kind="ExternalOutput")

    with tile.TileContext(nc) as tc:
        with (
            tc.tile_pool(name="persist", bufs=1) as pp,
            tc.tile_pool(name="vpool", bufs=1) as vp,
            tc.tile_pool(name="pj_ps", bufs=2, space="PSUM") as pj_ps,
            tc.tile_pool(name="st_ps", bufs=3, space="PSUM") as st_ps,
            tc.tile_pool(name="o_ps", bufs=2, space="PSUM") as o_ps_pool,
            tc.tile_pool(name="ptp", bufs=4) as ptp,
            tc.tile_pool(name="outp", bufs=3) as outp,
            tc.tile_pool(name="finp", bufs=3) as finp,
        ):
            # ---- persistent SBUF inputs
            xT = [pp.tile([128, t], BF16, name=f"xT{k}") for k in range(2)]
            xqT = [pp.tile([128, tq], BF16, name=f"xqT{k}") for k in range(2)]
            wq = [pp.tile([128, D], BF16, name=f"wq{k}") for k in range(2)]
            wk = [pp.tile([128, D], BF16, name=f"wk{k}") for k in range(2)]
            wv = [pp.tile([128, D], BF16, name=f"wv{k}") for k in range(2)]
            bq = [pp.tile([128, 1], F32, name=f"bq{k}") for k in range(2)]
            bk = [pp.tile([128, 1], F32, name=f"bk{k}") for k in range(2)]
            bvb = pp.tile([128, D], F32, name="bvb")
            mask = pp.tile([128, 256], F32, name="mask")
            for k in range(2):
                nc.sync.dma_start(xT[k][:], xT_d[k])
                nc.sync.dma_start(xqT[k][:], xqT_d[k])
                nc.sync.dma_start(wq[k][:], wq_d[k])
                nc.sync.dma_start(wk[k][:], wk_d[k])
                nc.sync.dma_start(wv[k][:], wv_d[k])
                nc.sync.dma_start(bq[k][:], bq_d[k])
                nc.sync.dma_start(bk[k][:], bk_d[k])
            nc.sync.dma_start(bvb[:], bvb_d[:])
            nc.sync.dma_start(mask[:], mask_d[:])

            # ---- projections: KT/QT in [dout, t] layout (bias via DVE)
            KT = [pp.tile([128, t], BF16, name=f"KT{m}") for m in range(2)]
            QT = [pp.tile([128, tq], BF16, name=f"QT{m}") for m in range(2)]
            NBK = min(512, t)
            NBQ = min(512, tq)
            for m in range(2):
                ms = slice(m * 128, (m + 1) * 128)
                for nb in range(t // NBK):
                    ps = pj_ps.tile([128, NBK], F32, name="pj", tag="pj")
                    for k in range(2):
                        nc.tensor.matmul(
                            ps[:],
                            wk[k][:, ms],
                            xT[k][:, nb * NBK : (nb + 1) * NBK],
                            start=(k == 0),
                            stop=(k == 1),
                        )
                    nc.vector.tensor_scalar_add(
                        KT[m][:, nb * NBK : (nb + 1) * NBK], ps[:], bk[m][:]
                    )
                for nb in range(tq // NBQ):
                    ps = pj_ps.tile([128, NBQ], F32, name="pj", tag="pj")
                    for k in range(2):
                        nc.tensor.matmul(
                            ps[:],
                            wq[k][:, ms],
                            xqT[k][:, nb * NBQ : (nb + 1) * NBQ],
                            start=(k == 0),
                            stop=(k == 1),
                        )
                    nc.vector.tensor_scalar_add(
                        QT[m][:, nb * NBQ : (nb + 1) * NBQ], ps[:], bq[m][:]
                    )

            # ---- V projection: natural [s, d] layout + ones column
            V = [vp.tile([128, D + 1], BF16, name=f"v{s}") for s in range(ns)]
            for s in range(ns):
                ps = pj_ps.tile([128, D], F32, name="pj", tag="pj")
                for k in range(2):
                    nc.tensor.matmul(
                        ps[:],
                        xT[k][:, s * 128 : (s + 1) * 128],
                        wv[k][:],
                        start=(k == 0),
                        stop=(k == 1),
                    )
                nc.vector.tensor_add(V[s][:, 0:D], ps[:], bvb[:])
                nc.vector.memset(V[s][:, D : D + 1], 1.0)

            # ---- attention
            exp_t = mybir.ActivationFunctionType.Exp
            for i in range(nq):
                e = 2 * i + 2  # s-tiles this query tile touches
                o_ps = o_ps_pool.tile([128, D + 1], F32, name="ops", tag="ops")
                qs = slice(i * 128, (i + 1) * 128)
                for s in range(e):
                    stp = st_ps.tile([128, 128], F32, name="stp", tag="stp")
                    for k in range(2):
                        nc.tensor.matmul(
                            stp[:],
                            KT[k][:, s * 128 : (s + 1) * 128],
                            QT[k][:, qs],
                            start=(k == 0),
                            stop=(k == 1),
                        )
                    if s == e - 2:
                        nc.vector.tensor_add(stp[:], stp[:], mask[:, 0:128])
                    elif s == e - 1:
                        nc.vector.tensor_add(stp[:], stp[:], mask[:, 128:256])
                    pt = ptp.tile([128, 128], BF16, name="pt", tag="pt")
                    nc.scalar.activation(pt[:], stp[:], exp_t, scale=scale)
                    nc.tensor.matmul(
                        o_ps[:], pt[:], V[s][:], start=(s == 0), stop=(s == e - 1)
                    )
                rec = finp.tile([128, 1], F32, name="rec", tag="rec", bufs=4)
                nc.vector.reciprocal(rec[:], o_ps[:, D : D + 1])
                ob = outp.tile([128, D], F32, name="ob", tag="ob")
                nc.vector.tensor_scalar_mul(ob[:], o_ps[:, 0:D], rec[:])
                nc.sync.dma_start(y_d[i * 128 : (i + 1) * 128, :], ob[:])
    return nc


def build_nc_v2(t: int = T, tq: int = TQ) -> bass.Bass:
    """Quad-grouped attention: 4 query tiles (512 q cols) share each S^T
    matmul / exp pass. Core h owns global q-tiles {8g + 2u + h}; group g
    runs a uniform s-extent of 8g+8 tiles on every core."""
    nq = tq // 128
    ns = t // 128
    ng = nq // 4
    assert t == 2 * tq and nq % 4 == 0
    scale = 1.0 / float(np.sqrt(np.float32(D)))

    nc = bacc.Bacc()
    xT_d = nc.dram_tensor("xT", [2, 128, t], BF16, kind="ExternalInput")
    xqT_d = nc.dram_tensor("xqT", [2, 128, tq], BF16, kind="ExternalInput")
    wq_d = nc.dram_tensor("wq", [2, 128, D], BF16, kind="ExternalInput")
    wk_d = nc.dram_tensor("wk", [2, 128, D], BF16, kind="ExternalInput")
    wv_d = nc.dram_tensor("wv", [2, 128, D], BF16, kind="ExternalInput")
    bq_d = nc.dram_tensor("bq", [2, 128, 1], F32, kind="ExternalInput")
    bk_d = nc.dram_tensor("bk", [2, 128, 1], F32, kind="ExternalInput")
    bvb_d = nc.dram_tensor("bvb", [128, D], F32, kind="ExternalInput")
    mask_d = nc.dram_tensor("mask", [8, 128, 512], F32, kind="ExternalInput")
    y_d = nc.dram_tensor("y", [tq, D], F32, kind="ExternalOutput")

    with tile.TileContext(nc) as tc:
        with (
            tc.tile_pool(name="persist", bufs=1) as pp,
            tc.tile_pool(name="vpool", bufs=1) as vp,
            tc.tile_pool(name="st_ps", bufs=2, space="PSUM") as st_ps,
            tc.tile_pool(name="o_ps", bufs=1, space="PSUM") as o_ps_pool,
            tc.tile_pool(name="ptp", bufs=3) as ptp,
            tc.tile_pool(name="outp", bufs=3) as outp,
            tc.tile_pool(name="finp", bufs=3) as finp,
        ):
            # ---- persistent SBUF inputs
            xT = [pp.tile([128, t], BF16, name=f"xT{k}") for k in range(2)]
            xqT = [pp.tile([128, tq], BF16, name=f"xqT{k}") for k in range(2)]
            wq = [pp.tile([128, D], BF16, name=f"wq{k}") for k in range(2)]
            wk = [pp.tile([128, D], BF16, name=f"wk{k}") for k in range(2)]
            wv = [pp.tile([128, D], BF16, name=f"wv{k}") for k in range(2)]
            bq = [pp.tile([128, 1], F32, name=f"bq{k}") for k in range(2)]
            bk = [pp.tile([128, 1], F32, name=f"bk{k}") for k in range(2)]
            bvb = pp.tile([128, D], F32, name="bvb")
            mask = [pp.tile([128, 512], F32, name=f"mask{r}") for r in range(8)]
            for k in range(2):
                nc.sync.dma_start(xT[k][:], xT_d[k])
                nc.sync.dma_start(xqT[k][:], xqT_d[k])
                nc.sync.dma_start(wq[k][:], wq_d[k])
                nc.sync.dma_start(wk[k][:], wk_d[k])
                nc.sync.dma_start(wv[k][:], wv_d[k])
                nc.sync.dma_start(bq[k][:], bq_d[k])
                nc.sync.dma_start(bk[k][:], bk_d[k])
            nc.sync.dma_start(bvb[:], bvb_d[:])
            for r in range(8):
                nc.sync.dma_start(mask[r][:], mask_d[r])

            KT = [pp.tile([128, t], BF16, name=f"KT{m}") for m in range(2)]
            QT = [pp.tile([128, tq], BF16, name=f"QT{m}") for m in range(2)]
            V = [vp.tile([128, D + 1], BF16, name=f"v{s}") for s in range(ns)]

            # ---- projections in their own PSUM pool (freed before attention)
            with tc.tile_pool(name="pj_ps", bufs=2, space="PSUM") as pj_ps:
                NBK = min(512, t)
                NBQ = min(512, tq)
                for m in range(2):
                    ms = slice(m * 128, (m + 1) * 128)
                    for nb in range(t // NBK):
                        ps = pj_ps.tile([128, NBK], F32, name="pj", tag="pj")
                        for k in range(2):
                            nc.tensor.matmul(
                                ps[:],
                                wk[k][:, ms],
                                xT[k][:, nb * NBK : (nb + 1) * NBK],
                                start=(k == 0),
                                stop=(k == 1),
                            )
                        nc.vector.tensor_scalar_add(
                            KT[m][:, nb * NBK : (nb + 1) * NBK], ps[:], bk[m][:]
                        )
                    for nb in range(tq // NBQ):
                        ps = pj_ps.tile([128, NBQ], F32, name="pj", tag="pj")
                        for k in range(2):
                            nc.tensor.matmul(
                                ps[:],
                                wq[k][:, ms],
                                xqT[k][:, nb * NBQ : (nb + 1) * NBQ],
                                start=(k == 0),
                                stop=(k == 1),
                            )
                        nc.vector.tensor_scalar_add(
                            QT[m][:, nb * NBQ : (nb + 1) * NBQ], ps[:], bq[m][:]
                        )
                for s in range(ns):
                    ps = pj_ps.tile([128, D], F32, name="pj", tag="pj")
                    for k in range(2):
                        nc.tensor.matmul(
                            ps[:],
                            xT[k][:, s * 128 : (s + 1) * 128],
                            wv[k][:],
                            start=(k == 0),
                            stop=(k == 1),
                        )
                    nc.vector.tensor_add(V[s][:, 0:D], ps[:], bvb[:])
                    nc.vector.memset(V[s][:, D : D + 1], 1.0)

            # ---- attention, 512 q cols per group
            exp_t = mybir.ActivationFunctionType.Exp
            att = ctx_att = tc.tile_pool(name="st_ps", bufs=2, space="PSUM")
            st_ps = att.__enter__()
            o_ctx = tc.tile_pool(name="o_ps", bufs=6, space="PSUM")
            o_ps_pool = o_ctx.__enter__()
            for g in range(ng):
                e = 8 * g + 8
                qs = slice(g * 512, (g + 1) * 512)
                o = [
                    o_ps_pool.tile([128, D + 1], F32, name=f"o{u}", tag=f"o{u}")
                    for u in range(4)
                ]
                for s in range(e):
                    stp = st_ps.tile([128, 512], F32, name="stp", tag="stp")
                    for k in range(2):
                        nc.tensor.matmul(
                            stp[:],
                            KT[k][:, s * 128 : (s + 1) * 128],
                            QT[k][:, qs],
                            start=(k == 0),
                            stop=(k == 1),
                        )
                    if s >= 8 * g:
                        nc.vector.tensor_add(stp[:], stp[:], mask[s - 8 * g][:])
                    pt = ptp.tile([128, 512], BF16, name="pt", tag="pt")
                    nc.scalar.activation(pt[:], stp[:], exp_t, scale=scale)
                    for u in range(4):
                        nc.tensor.matmul(
                            o[u][:],
                            pt[:, u * 128 : (u + 1) * 128],
                            V[s][:],
                            start=(s == 0),
                            stop=(s == e - 1),
                        )
                for u in range(4):
                    rec = finp.tile([128, 1], F32, name="rec", tag="rec", bufs=4)
                    nc.vector.reciprocal(rec[:], o[u][:, D : D + 1])
                    ob = outp.tile([128, D], F32, name="ob", tag="ob")
                    nc.vector.tensor_scalar_mul(ob[:], o[u][:, 0:D], rec[:])
                    lrow = (g * 4 + u) * 128
                    nc.sync.dma_start(y_d[lrow : lrow + 128, :], ob[:])
    return nc


def build_nc_v3(t: int = T, tq: int = TQ, st_bufs: int = 4, o_bufs: int = 4, pt_bufs: int = 6, pj_bufs: int = 4) -> bass.Bass:
    """v2 + cheaper masking, less dead work, and walrus-friendly syncs:
    - all constants (weights, biases, masks) packed into two DRAM tensors
      loaded with one DMA each; tiny DVE "absorber" copies pull the DMA
      completion into DVE's vector clock so the bias TensorScalarPtr ops
      carry a single sem wait (walrus rejects multi-wait TS instrs);
    - causal mask applied AFTER exp as a multiplicative 0/1 bf16 mask on
      one 128-col block per diagonal s-tile (DVE bf16 SBUF fast mode);
    - PV matmuls skipped for (s_rel, u) tiles dead on BOTH cores
      (u < floor(s_rel/2)) — the skip pattern is SPMD-uniform;
    - input x DMAs chunked so projections overlap the loads;
    - single-tag o-pool (bufs=6) so group g+1 does not wait on group g's
      finalize."""
    nq = tq // 128
    ns = t // 128
    ng = nq // 4
    assert t == 2 * tq and nq % 4 == 0
    scale = 1.0 / float(np.sqrt(np.float32(D)))

    nc = bacc.Bacc()
    xT_d = nc.dram_tensor("xT", [2, 128, t // 128, 128], BF16, kind="ExternalInput")
    cw_d = nc.dram_tensor("cw", [128, 1536], BF16, kind="ExternalInput")
    cm_d = nc.dram_tensor("cm", [128, 1024], BF16, kind="ExternalInput")
    cf_d = nc.dram_tensor("cf", [128, 260], F32, kind="ExternalInput")
    y_d = nc.dram_tensor("y", [tq, D], BF16, kind="ExternalOutput")

    with tile.TileContext(nc) as tc:
        with (
            tc.tile_pool(name="persist", bufs=1) as pp,
            tc.tile_pool(name="vpool", bufs=1) as vp,
            tc.tile_pool(name="ptp", bufs=pt_bufs) as ptp,
            tc.tile_pool(name="outp", bufs=3) as outp,
            tc.tile_pool(name="finp", bufs=4) as finp,
        ):
            # ---- inputs. One sync (HWDGE) queue so transfers complete in
            # priority order: weights -> first x chunks (gates the first
            # projection matmuls) -> rest -> masks (needed ~20us in).
            cw = pp.tile([128, 1536], BF16, name="cw")
            cm = pp.tile([128, 1024], BF16, name="cm")
            cf = pp.tile([128, 260], F32, name="cf")
            xT = [pp.tile([128, t], BF16, name=f"xT{k}") for k in range(2)]
            xqT = [pp.tile([128, tq], BF16, name=f"xqT{k}") for k in range(2)]
            CH = max(512, t // 2)
            nc.sync.dma_start(cw[:], cw_d[:])
            for k in range(2):
                nc.sync.dma_start(xT[k][:, 0:CH], xT_d[k][:, 0:CH])
            nc.sync.dma_start(cf[:], cf_d[:])
            for c0 in range(CH, t, CH):
                for k in range(2):
                    nc.sync.dma_start(xT[k][:, c0 : c0 + CH], xT_d[k][:, c0 : c0 + CH])
            for k in range(2):
                nc.sync.dma_start(xqT[k][:], xqT_d[k])
            nc.sync.dma_start(cm[:], cm_d[:])
            # absorber copies: pull each const DMA's completion into DVE's
            # vector clock so downstream DVE ops carry a single sem wait
            scrb = finp.tile([128, 1], BF16, name="scrb", tag="scrb")
            nc.vector.tensor_copy(scrb[:], cw[:, 0:1])
            scrf = finp.tile([128, 1], F32, name="scrf", tag="scrf", bufs=1)
            nc.vector.tensor_copy(scrf[:], cf[:, 0:1])
            scrm = finp.tile([128, 1], BF16, name="scrm", tag="scrm", bufs=1)
            nc.vector.tensor_copy(scrm[:], cm[:, 0:1])
            wq = [cw[:, 0 + k * 256 : 256 + k * 256] for k in range(2)]
            wk = [cw[:, 512 + k * 256 : 768 + k * 256] for k in range(2)]
            wv = [cw[:, 1024 + k * 256 : 1280 + k * 256] for k in range(2)]
            maskb = [cm[:, r * 128 : (r + 1) * 128] for r in range(8)]
            bq = [cf[:, k : k + 1] for k in range(2)]
            bk = [cf[:, 2 + k : 3 + k] for k in range(2)]
            bvb = cf[:, 4:260]

            # HAM warm-up: garbage matmuls while input DMAs land, so the
            # PE clock gate is already at 8/8 when real work arrives.
            wa = pp.tile([128, 128], BF16, name="wa")
            wb = pp.tile([128, 512], BF16, name="wb")
            nc.vector.memset(wa[:], 0.0)
            nc.vector.memset(wb[:], 0.0)
            with tc.tile_pool(name="warm_ps", bufs=1, space="PSUM") as wps:
                wp_t = wps.tile([128, 512], F32, name="warm")
                for _ in range(20):
                    nc.tensor.matmul(wp_t[:], wa[:], wb[:], start=True, stop=True)

            KT = [pp.tile([128, t], BF16, name=f"KT{m}") for m in range(2)]
            QT = [pp.tile([128, tq], BF16, name=f"QT{m}") for m in range(2)]
            V = [vp.tile([128, D + 1], BF16, name=f"v{s}") for s in range(ns)]

            # ---- projections in their own PSUM pool (freed before attention)
            with tc.tile_pool(name="pj_ps", bufs=pj_bufs, space="PSUM") as pj_ps:
                NBK = min(512, t)
                NBQ = min(512, tq)
                for nb in range(t // NBK):
                    for m in range(2):
                        ms = slice(m * 128, (m + 1) * 128)
                        ps = pj_ps.tile([128, NBK], F32, name="pj", tag="pj")
                        for k in range(2):
                            nc.tensor.matmul(
                                ps[:],
                                wk[k][:, ms],
                                xT[k][:, nb * NBK : (nb + 1) * NBK],
                                start=(k == 0),
                                stop=(k == 1),
                            )
                        nc.vector.tensor_scalar_add(
                            KT[m][:, nb * NBK : (nb + 1) * NBK], ps[:], bk[m]
                        )
                for m in range(2):
                    ms = slice(m * 128, (m + 1) * 128)
                    for nb in range(tq // NBQ):
                        ps = pj_ps.tile([128, NBQ], F32, name="pj", tag="pj")
                        for k in range(2):
                            nc.tensor.matmul(
                                ps[:],
                                wq[k][:, ms],
                                xqT[k][:, nb * NBQ : (nb + 1) * NBQ],
                                start=(k == 0),
                                stop=(k == 1),
                            )
                        nc.vector.tensor_scalar_add(
                            QT[m][:, nb * NBQ : (nb + 1) * NBQ], ps[:], bq[m]
                        )

            # ---- attention, 512 q cols per group
            exp_t = mybir.ActivationFunctionType.Exp
            att = ctx_att = tc.tile_pool(name="st_ps", bufs=st_bufs, space="PSUM")
            st_ps = att.__enter__()
            o_ctx = tc.tile_pool(name="o_ps", bufs=o_bufs, space="PSUM")
            o_ps_pool = o_ctx.__enter__()
            for g in range(ng):
                e = 8 * g + 8
                for s in range(8 * g, min(8 * g + 8, ns)):
                    ps = st_ps.tile([128, D], F32, name="vpj", tag="stp")
                    for k in range(2):
                        nc.tensor.matmul(
                            ps[:],
                            xT[k][:, s * 128 : (s + 1) * 128],
                            wv[k],
                            start=(k == 0),
                            stop=(k == 1),
                        )
                    # bias-free V: since sum_s P = den, (O + den*bv)/den =
                    # O/den + bv, so bv moves to the finalize and this
                    # PSUM->SBUF cast-copy runs on the idle ACT engine
                    nc.scalar.copy(V[s][:, 0:D], ps[:])
                    nc.vector.memset(V[s][:, D : D + 1], 1.0)
                qs = slice(g * 512, (g + 1) * 512)
                o = [
                    o_ps_pool.tile([128, D + 1], F32, name=f"o{u}", tag="o")
                    for u in range(4)
                ]
                for s in range(e):
                    s_rel = s - 8 * g
                    u0 = max(s_rel, 0) // 2  # first live 128-col block
                    c0 = u0 * 128
                    stp = st_ps.tile([128, 512], F32, name="stp", tag="stp")
                    for k in range(2):
                        nc.tensor.matmul(
                            stp[:, c0:512],
                            KT[k][:, s * 128 : (s + 1) * 128],
                            QT[k][:, g * 512 + c0 : (g + 1) * 512],
                            start=(k == 0),
                            stop=(k == 1),
                        )
                    pt = ptp.tile([128, 512], BF16, name="pt", tag="pt")
                    nc.scalar.activation(
                        pt[:, c0:512], stp[:, c0:512], exp_t, scale=scale
                    )
                    if s_rel >= 0:
                        nc.vector.tensor_mul(
                            pt[:, c0 : c0 + 128],
                            pt[:, c0 : c0 + 128],
                            maskb[s_rel],
                        )
                    for u in range(4):
                        if s_rel >= 0 and u < s_rel // 2:
                            continue  # dead on every core
                        nc.tensor.matmul(
                            o[u][:],
                            pt[:, u * 128 : (u + 1) * 128],
                            V[s][:],
                            start=(s == 0),
                            stop=(s == 8 * g + 2 * u + 1),
                        )
                for u in range(4):
                    rec = finp.tile([128, 1], F32, name="rec", tag="rec", bufs=4)
                    nc.vector.reciprocal(rec[:], o[u][:, D : D + 1])
                    ob = outp.tile([128, D], F32, name="ob", tag="ob")
                    nc.vector.scalar_tensor_tensor(
                        ob[:],
                        o[u][:, 0:D],
                        rec[:],
                        bvb,
                        mybir.AluOpType.mult,
                        mybir.AluOpType.add,
                    )
                    lrow = (g * 4 + u) * 128
                    nc.sync.dma_start(y_d[lrow : lrow + 128, :], ob[:])
            o_ctx.__exit__(None, None, None)
            ctx_att.__exit__(None, None, None)
    return nc


def build_nc_v4(
    t: int = T,
    tq: int = TQ,
    st_bufs: int = 2,
    o_bufs: int = 4,
    pt_bufs: int = 4,
    warm: int = 11,
) -> bass.Bass:
    """v3 + fp8 attention logits and a restructured schedule:
    - QK^T runs as ONE fp8e4 DoubleRow matmul per s-tile (contract 256 in
      216ns, 2x bf16 FLOP rate). KT/QT are written directly as fp8 by the
      projection PSUM->SBUF copies (no extra cast pass). K carries no bias:
      (q+bq)@(k+bk) == (q+bq)@k modulo a per-query constant, and softmax is
      shift-invariant, so dropping bk is exact.
    - s-tiles processed in PAIRS sharing a [128,2,512] 2-bank PSUM tile and
      ONE wide exp per pair (halves ACT instruction count; ACT would
      otherwise become the bottleneck once S^T is fp8).
    - all projections upfront, dependency-ordered against the input DMA
      stream; PSUM->SBUF copies split between DVE and ACT so neither trails
      the PE. V is projected in quads (4 s-tiles per 2-bank PSUM tile).
    - o[u] finalized EARLY (right after its stop matmul) so the o-pool
      recycles promptly and the kernel tail is short.
    - input DMA issue split across SP (cw, xT) / ACT (cf, xqT) / GPSIMD
      (cm) queues so descriptor-generation overhead overlaps."""
    nq = tq // 128
    ns = t // 128
    ng = nq // 4
    assert t == 2 * tq and nq % 4 == 0
    scale = 1.0 / float(np.sqrt(np.float32(D)))
    FP8 = mybir.dt.float8e4
    DR = mybir.MatmulPerfMode.DoubleRow

    nc = bacc.Bacc()
    xT_d = nc.dram_tensor("xT", [2, 128, t // 128, 128], BF16, kind="ExternalInput")
    cw_d = nc.dram_tensor("cw", [128, 1536], BF16, kind="ExternalInput")
    cm_d = nc.dram_tensor("cm", [128, 1024], BF16, kind="ExternalInput")
    cf_d = nc.dram_tensor("cf", [128, 260], F32, kind="ExternalInput")
    y_d = nc.dram_tensor("y", [tq, D], BF16, kind="ExternalOutput")

    with tile.TileContext(nc) as tc:
        with (
            tc.tile_pool(name="persist", bufs=1) as pp,
            tc.tile_pool(name="vpool", bufs=1) as vp,
            tc.tile_pool(name="ptp", bufs=pt_bufs) as ptp,
            tc.tile_pool(name="outp", bufs=2) as outp,
            tc.tile_pool(name="finp", bufs=2) as finp,
            tc.tile_pool(name="ps", bufs=1, space="PSUM") as psp,
        ):
            cw = pp.tile([128, 1536], BF16, name="cw")
            cm = pp.tile([128, 1024], BF16, name="cm")
            cf = pp.tile([128, 260], F32, name="cf")
            xT = [pp.tile([128, t // 128, 128], BF16, name=f"xT{k}") for k in range(2)]
            # Two issue queues: SP carries wk + the k=0 chunks, ACT carries
            # cf + the k=1 chunks, both in consumption order.
            nc.sync.dma_start(cw[:, 512:1024], cw_d[:, 512:1024])
            nc.scalar.dma_start(cf[:], cf_d[:])
            nc.sync.dma_start(xT[0][:, 0:4, :], xT_d[0][:, 0:4])
            nc.scalar.dma_start(xT[1][:, 0:4, :], xT_d[1][:, 0:4])
            nc.sync.dma_start(xT[0][:, 4:8, :], xT_d[0][:, 4:8])
            nc.scalar.dma_start(xT[1][:, 4:8, :], xT_d[1][:, 4:8])
            nc.sync.dma_start(cw[:, 0:512], cw_d[:, 0:512])
            nc.scalar.dma_start(cw[:, 1024:1536], cw_d[:, 1024:1536])
            for a in range(8, t // 128, 8):
                nc.sync.dma_start(xT[0][:, a : a + 8, :], xT_d[0][:, a : a + 8])
                nc.scalar.dma_start(xT[1][:, a : a + 8, :], xT_d[1][:, a : a + 8])
            # GPSIMD queue: masks (needed ~20us in)
            nc.gpsimd.dma_start(cm[:], cm_d[:])
            # absorber copies: fold const DMA completions into DVE's clock
            scrf = finp.tile([128, 1], F32, name="scrf", tag="scrf", bufs=1)
            nc.vector.tensor_copy(scrf[:], cf[:, 0:1])
            scrm = finp.tile([128, 1], BF16, name="scrm", tag="scrm", bufs=1)
            nc.vector.tensor_copy(scrm[:], cm[:, 0:1])
            wq = [cw[:, 0 + k * 256 : 256 + k * 256] for k in range(2)]
            wk = [cw[:, 512 + k * 256 : 768 + k * 256] for k in range(2)]
            wv = [cw[:, 1024 + k * 256 : 1280 + k * 256] for k in range(2)]
            maskb = [cm[:, r * 128 : (r + 1) * 128] for r in range(8)]
            bq = [cf[:, k : k + 1] for k in range(2)]
            zb = cf[:, 2:3]  # zeros column (K has no bias)
            bvb = cf[:, 4:260]

            # HAM warm-up while the first input DMAs land
            wa = pp.tile([128, 128], BF16, name="wa")
            wb = pp.tile([128, 512], BF16, name="wb")
            nc.vector.memset(wa[:], 0.0)
            nc.vector.memset(wb[:], 0.0)
            for _ in range(warm):
                wp_t = psp.tile([128, 2, 512], F32, name="warm", tag="stp", bufs=st_bufs)
                nc.tensor.matmul(
                    wp_t[:, 0, :], wa[:], wb[:], start=True, stop=True
                )

            KT8 = pp.tile([128, 2, t], FP8, name="KT8")
            QT8 = pp.tile([128, 2, tq], FP8, name="QT8")
            VQ = vp.tile([128, ns, D + 1], BF16, name="VQ")

            # ---- projections, interleaved in DMA-arrival order.
            # Each item is ONE single-bank PSUM tile (2-4 matmuls + 1
            # PSUM->SBUF cast copy). Tiles rotate over all 6 PSUM slots
            # (4 "o" banks + 2 double-bank "stp" slots) and the copies
            # alternate DVE/ACT, so the PE never waits on a copy.
            pj_cnt = [0]

            def pj_tile():
                i = pj_cnt[0]
                pj_cnt[0] += 1
                if i % 3 == 2:
                    return psp.tile(
                        [128, 512], F32, name="pjs", tag="stp", bufs=st_bufs
                    )
                return psp.tile([128, 512], F32, name="pjo", tag="o", bufs=o_bufs)

            def on_dve():
                return pj_cnt[0] % 2 == 0

            def k_half(nb, m):
                ps = pj_tile()
                cs = slice(nb * 512, (nb + 1) * 512)
                for k in range(2):
                    nc.tensor.matmul(
                        ps[:],
                        wk[k][:, m * 128 : (m + 1) * 128],
                        xT[k][:, 4 * nb : 4 * nb + 4, :],
                        start=(k == 0),
                        stop=(k == 1),
                    )
                if on_dve():
                    nc.vector.tensor_scalar_add(KT8[:, m, cs], ps[:], zb)
                else:
                    nc.scalar.copy(KT8[:, m, cs], ps[:])

            def q_half(nb, m):
                # local q tiles 4*nb..4*nb+3 live at even positions of the
                # (possibly host-pair-swapped) xT tile axis
                ps = pj_tile()
                cs = slice(nb * 512, (nb + 1) * 512)
                for k in range(2):
                    nc.tensor.matmul(
                        ps[:],
                        wq[k][:, m * 128 : (m + 1) * 128],
                        xT[k][:, 8 * nb : 8 * nb + 8 : 2, :],
                        start=(k == 0),
                        stop=(k == 1),
                    )
                if on_dve():
                    nc.vector.tensor_scalar_add(QT8[:, m, cs], ps[:], bq[m])
                else:
                    nc.scalar.activation(
                        QT8[:, m, cs],
                        ps[:],
                        mybir.ActivationFunctionType.Identity,
                        bias=bq[m],
                    )

            def v_pair(p):
                ps = pj_tile()
                for j in range(2):
                    for k in range(2):
                        nc.tensor.matmul(
                            ps[:, j * 256 : (j + 1) * 256],
                            xT[k][:, 2 * p + j, :],
                            wv[k][:],
                            start=(k == 0),
                            stop=(k == 1),
                        )
                src = ps[:].rearrange("p (a c) -> p a c", a=2)
                dst = VQ[:, 2 * p : 2 * p + 2, 0:D]
                if on_dve():
                    nc.vector.tensor_copy(dst, src)
                else:
                    nc.scalar.copy(dst, src)
                nc.vector.memset(VQ[:, 2 * p : 2 * p + 2, D : D + 1], 1.0)

            for nb in range(t // 512):
                k_half(nb, 0)
                k_half(nb, 1)
                if nb % 2 == 1:
                    q_half(nb // 2, 0)
                    q_half(nb // 2, 1)
                    v_pair(2 * (nb - 1))
                    v_pair(2 * nb - 1)
                    v_pair(2 * nb)
                    v_pair(2 * nb + 1)

            # ---- attention: s-tile pairs, one DoubleRow S^T per s-tile,
            # one wide exp per pair. 1-stage software pipeline: S^T+exp of
            # pair i+1 is emitted BEFORE the PV block of pair i, so the exp
            # latency is always hidden behind PE work (incl. across group
            # boundaries).
            exp_t = mybir.ActivationFunctionType.Exp
            # last two diag pairs of each group (widths 256+128 per j) merge
            # into ONE item sharing one stp tile and ONE exp: the narrow
            # pair's S^T lands relocated at cols [128:256] (its PV/mask
            # slices are adjusted to match), halving ACT instruction
            # overhead exactly where the group tail is ACT-bound
            items = []
            for g in range(ng):
                for p in range(4 * g + 2):
                    items.append((g, p, False))
                items.append((g, 4 * g + 2, True))
            state = {}  # per-group o tiles / ob4

            def stage_a(i):
                g, p, merged = items[i]
                if merged:
                    stp = psp.tile(
                        [128, 2, 512], F32, name="stp", tag="stp", bufs=st_bufs
                    )
                    for j in range(2):  # pair-a (d0=4): blocks 2,3 in place
                        s = 2 * p + j
                        nc.tensor.matmul(
                            stp[:, j, 256:512],
                            KT8[:, :, s * 128 : (s + 1) * 128],
                            QT8[:, :, g * 512 + 256 : (g + 1) * 512],
                            start=True,
                            stop=True,
                            perf_mode=DR,
                        )
                    for j in range(2):  # pair-b (d0=6): block 3 relocated
                        s = 2 * p + 2 + j
                        nc.tensor.matmul(
                            stp[:, j, 128:256],
                            KT8[:, :, s * 128 : (s + 1) * 128],
                            QT8[:, :, g * 512 + 384 : (g + 1) * 512],
                            start=True,
                            stop=True,
                            perf_mode=DR,
                        )
                    pt = ptp.tile([128, 2, 512], BF16, name="pt", tag="pt")
                    nc.scalar.activation(
                        pt[:, :, 128:512], stp[:, :, 128:512], exp_t, scale=scale
                    )
                    m2a = cm[:, 4 * 128 : 6 * 128].rearrange(
                        "p (a c) -> p a c", a=2
                    )
                    nc.vector.tensor_mul(
                        pt[:, :, 256:384], pt[:, :, 256:384], m2a
                    )
                    m2b = cm[:, 6 * 128 : 8 * 128].rearrange(
                        "p (a c) -> p a c", a=2
                    )
                    nc.vector.tensor_mul(
                        pt[:, :, 128:256], pt[:, :, 128:256], m2b
                    )
                    return pt
                s0 = 2 * p
                d0 = s0 - 8 * g  # s_rel of j=0 (even; may be < 0)
                u0 = max(d0, 0) // 2
                c0 = u0 * 128
                stp = psp.tile(
                    [128, 2, 512], F32, name="stp", tag="stp", bufs=st_bufs
                )
                for j in range(2):
                    s = s0 + j
                    nc.tensor.matmul(
                        stp[:, j, c0:512],
                        KT8[:, :, s * 128 : (s + 1) * 128],
                        QT8[:, :, g * 512 + c0 : (g + 1) * 512],
                        start=True,
                        stop=True,
                        perf_mode=DR,
                    )
                pt = ptp.tile([128, 2, 512], BF16, name="pt", tag="pt")
                nc.scalar.activation(
                    pt[:, :, c0:512], stp[:, :, c0:512], exp_t, scale=scale
                )
                if d0 >= 0:
                    m2 = cm[:, d0 * 128 : (d0 + 2) * 128].rearrange(
                        "p (a c) -> p a c", a=2
                    )
                    nc.vector.tensor_mul(
                        pt[:, :, c0 : c0 + 128], pt[:, :, c0 : c0 + 128], m2
                    )
                return pt

            def stage_b(i, pt):
                g, p, merged = items[i]
                if merged:
                    o = state["o"]
                    ob4 = state["ob4"]
                    for j in range(2):  # pair-a: u=2 (diag) and u=3
                        s = 2 * p + j
                        for u in (2, 3):
                            nc.tensor.matmul(
                                o[u][:],
                                pt[:, j, u * 128 : (u + 1) * 128],
                                VQ[:, s, :],
                                start=False,
                                stop=(s == 8 * g + 2 * u + 1),
                            )
                            if j == 1 and u == 2:
                                finalize(2)
                    for j in range(2):  # pair-b: u=3 only, relocated cols
                        s = 2 * p + 2 + j
                        nc.tensor.matmul(
                            o[3][:],
                            pt[:, j, 128:256],
                            VQ[:, s, :],
                            start=False,
                            stop=(s == 8 * g + 7),
                        )
                        if j == 1:
                            finalize(3)
                    return
                if p == 0:
                    state["o"] = [
                        psp.tile(
                            [128, D + 1], F32, name=f"o{u}", tag="o", bufs=o_bufs
                        )
                        for u in range(4)
                    ]
                    state["ob4"] = outp.tile([128, 4, D], BF16, name="ob4", tag="ob")
                o = state["o"]
                ob4 = state["ob4"]
                s0 = 2 * p
                d0 = s0 - 8 * g
                # j-major PE order (u-major measurably slows every matmul);
                # the diagonal u0's finalize is emitted right after its stop
                # matmul (j=1, u=u0) so the o slot frees ~3 PVs earlier
                def finalize(u):
                    rec = finp.tile([128, 1], F32, name="rec", tag="rec", bufs=4)
                    nc.vector.reciprocal(rec[:], o[u][:, D : D + 1])
                    nc.vector.scalar_tensor_tensor(
                        ob4[:, u, :],
                        o[u][:, 0:D],
                        rec[:],
                        bvb,
                        mybir.AluOpType.mult,
                        mybir.AluOpType.add,
                    )
                    if g == ng - 1:
                        # last group: per-u stores on the (now idle) ACT
                        # queue so the final DMA payload is small and SP's
                        # teardown overlaps the issue
                        lrow = (g * 4 + u) * 128
                        nc.scalar.dma_start(
                            y_d[lrow : lrow + 128, :], ob4[:, u, :]
                        )
                    elif u == 3:
                        # batched y store for the whole group
                        dst = y_d[g * 512 : (g + 1) * 512, :].rearrange(
                            "(u p) d -> p u d", u=4
                        )
                        nc.sync.dma_start(dst, ob4[:])

                for j in range(2):
                    s = s0 + j
                    d = d0 + j
                    for u in range(4):
                        if d >= 0 and u < d // 2:
                            continue  # dead on every core
                        nc.tensor.matmul(
                            o[u][:],
                            pt[:, j, u * 128 : (u + 1) * 128],
                            VQ[:, s, :],
                            start=(s == 0),
                            stop=(s == 8 * g + 2 * u + 1),
                        )
                        if j == 1 and d0 >= 0 and u == d0 // 2:
                            finalize(u)

            prev = stage_a(0)
            for i in range(1, len(items)):
                cur = stage_a(i)
                stage_b(i - 1, prev)
                prev = cur
            stage_b(len(items) - 1, prev)
    return nc


def prep_inputs(
    x, Wq, bq, Wk, bk, Wv, bv, t: int = T, n_cores: int = N_CORES, version: int = 1
):
    """Per-core input maps (host-side shard / transpose / cast)."""
    x = np.asarray(x, dtype=np.float32)
    b_dim = x.shape[0]
    tq = t // 2
    nq = tq // 128
    shared = {}
    for name, w in (("wq", Wq), ("wk", Wk), ("wv", Wv)):
        shared[name] = np.ascontiguousarray(
            np.asarray(w, np.float32).astype(NPBF16).reshape(2, 128, D)
        )
    shared["bq"] = np.ascontiguousarray(
        np.asarray(bq, np.float32).reshape(2, 128, 1)
    )
    shared["bk"] = np.ascontiguousarray(
        np.asarray(bk, np.float32).reshape(2, 128, 1)
    )
    shared["bvb"] = np.ascontiguousarray(
        np.broadcast_to(np.asarray(bv, np.float32), (128, D))
    )
    idx = np.arange(128)
    tri = np.where(idx[:, None] > idx[None, :], np.float32(NEG), np.float32(0.0))
    full = np.full((128, 128), NEG, np.float32)
    zero = np.zeros((128, 128), np.float32)
    if version == 1:
        masks = [
            np.ascontiguousarray(np.concatenate([tri, full], axis=1)),
            np.ascontiguousarray(np.concatenate([zero, tri], axis=1)),
        ]
    elif version == 2:
        masks = []
        for h in range(2):
            m = np.empty((8, 128, 512), np.float32)
            for s_rel in range(8):
                for u in range(4):
                    blk = full if s_rel > 2 * u + h else (tri if s_rel == 2 * u + h else zero)
                    m[s_rel, :, u * 128 : (u + 1) * 128] = blk
            masks.append(np.ascontiguousarray(m))
    else:
        # v3: multiplicative 0/1 bf16 masks, one 128-block per diag s_rel.
        # s_rel even -> block u0=s_rel/2: h=0 diag (keep s<=q), h=1 keep-all
        # s_rel odd  -> block u0:         h=0 dead (zeros),     h=1 diag
        # v4: core h=1 gets its x tiles PAIR-SWAPPED on host, so position
        # s_rel even holds the diag tile on BOTH cores; odd positions are
        # dead (h=0) / keep-all (h=1).
        tri01 = (idx[:, None] <= idx[None, :]).astype(NPBF16)
        ones = np.ones((128, 128), NPBF16)
        zeros = np.zeros((128, 128), NPBF16)
        masks = []
        for h in range(2):
            m = np.empty((8, 128, 128), NPBF16)
            for s_rel in range(8):
                if s_rel % 2 == 0:
                    m[s_rel] = (
                        tri01 if (h == 0 or version >= 4) else ones
                    )
                else:
                    m[s_rel] = zeros if h == 0 else (
                        ones if version >= 4 else tri01
                    )
            masks.append(m)
        # pack constants: cb [128, 2560] bf16, cf [128, 260] f32
        cf = np.empty((128, 260), np.float32)
        cf[:, 0:2] = np.asarray(bq, np.float32).reshape(2, 128).T
        if version >= 4:
            # v4 drops bk entirely (softmax shift-invariance); col 2 must
            # be zero (used as the zero-bias for the K fp8 write-out)
            cf[:, 2:4] = 0.0
        else:
            cf[:, 2:4] = np.asarray(bk, np.float32).reshape(2, 128).T
        cf[:, 4:260] = np.broadcast_to(np.asarray(bv, np.float32), (128, D))
        cw = np.empty((128, 1536), NPBF16)
        for j, w in enumerate((Wq, Wk, Wv)):
            wb = np.asarray(w, np.float32).astype(NPBF16).reshape(2, 128, D)
            cw[:, j * 512 : j * 512 + 256] = wb[0]
            cw[:, j * 512 + 256 : j * 512 + 512] = wb[1]
        cms = []
        for h in range(2):
            cm = np.empty((128, 1024), NPBF16)
            for r in range(8):
                cm[:, r * 128 : (r + 1) * 128] = masks[h][r]
            cms.append(np.ascontiguousarray(cm))
        shared = {"cf": np.ascontiguousarray(cf), "cw": np.ascontiguousarray(cw)}
    in_maps = []
    for c in range(n_cores):
        b, h = divmod(c, 2)
        xb = x[b % b_dim]  # [t, D]
        if version >= 4:
            xt4 = xb.T.astype(NPBF16).reshape(2, 128, t // 128, 128)
            if h == 1:
                swap = np.arange(t // 128) ^ 1  # pair-swap the tile axis
                xt4 = xt4[:, :, swap, :]
            in_maps.append(
                {"xT": np.ascontiguousarray(xt4), "cm": cms[h], **shared}
            )
            continue
        xT = np.ascontiguousarray(xb.T.astype(NPBF16).reshape(2, 128, t))
        qrows = np.concatenate(
            [xb[g * 128 : (g + 1) * 128] for g in _qtiles(nq, h, version)], axis=0
        )
        xqT = np.ascontiguousarray(qrows.T.astype(NPBF16).reshape(2, 128, tq))
        if version >= 3:
            in_maps.append({"xT": xT, "xqT": xqT, "cm": cms[h], **shared})
        else:
            in_maps.append({"xT": xT, "xqT": xqT, "mask": masks[h], **shared})
    return in_maps


def _qtiles(nq: int, h: int, version: int) -> list[int]:
    """Global q-tile index for each local tile, in local order."""
    if version == 1:
        return [2 * i + h for i in range(nq)]
    return [8 * g + 2 * u + h for g in range(nq // 4) for u in range(4)]


_BUILDERS = {1: build_nc, 2: build_nc_v2, 3: build_nc_v3, 4: build_nc_v4}


def gather_output(results, t: int = T, n_cores: int = N_CORES, version: int = 1):
    tq = t // 2
    nq = tq // 128
    y = np.empty((n_cores // 2, t, D), np.float32)
    for c in range(n_cores):
        b, h = divmod(c, 2)
        yc = np.asarray(results[c]["y"]).astype(np.float32)
        for li, g in enumerate(_qtiles(nq, h, version)):
            y[b, g * 128 : (g + 1) * 128] = yc[li * 128 : (li + 1) * 128]
    return y


VERSION = 4


def run_on_hw(inputs: dict, trace: bool = False):
    """Returns (y [B,T,D] f32, BassKernelResults)."""
    in_maps = prep_inputs(**inputs, version=VERSION)
    nc = _BUILDERS[VERSION]()
    if not nc.is_finalized():
        nc.finalize()
    res = run_bass_kernel_spmd(nc, in_maps, list(range(N_CORES)), trace=trace)
    return gather_output(res.results, version=VERSION), res


def kernel(**inputs) -> np.ndarray:
    y, _ = run_on_hw(inputs, trace=False)
    return y

